# revision 30
# baseline (speedup 1.0000x reference)
"""Trainium2 Bass kernel for nn_AxisSimplestSpline (saturation basis, col-tiled
PE, software-pipelined).

Math (per batch b, axis a):  f = A^T raw;  g = (f - mins)/dx in [0,17)
Saturation basis:  est_a(g) = Y0_a + s16_a*g + sum_{j=1..16} w_aj * min(g, j),
w_aj = s_{a,j-1} - s_{a,j}.  Every DVE feature is a single-ALU min ->
4x perf mode (203ns/512).  ACT knots use relu(j - g) = j - min(g,j)
(weight negated, j-term folded into host bias).  g is shifted by -8.5
(gc in [-8.5, 8.5]) to halve feature magnitudes (PE fp16 stream precision
scales with |value|).  All features derive from the SAME fp16-rounded gc
(integer-offset min/relu of fp16 is exact) so rounding errors are
perfectly correlated: total err ~ est(g~)-est(g).
Combine: 17 fp16 matmuls PSUM-accumulated on two concurrent 128x64 array
tiles (even emission -> partitions 0:48, odd -> 64:112); two rhs streams
flow at once (~157ns/512-col MM measured).  Proj matmuls padded to 96
contraction rows to keep tile_size uniform (no PE mode drains).
Software pipeline per 512-col chunk c:
  dma(c+2) | proj(c+1) | ACT g~(c+1) | DVE mins(c) | ACT relus(c)
  | knot MMs(c) | ACT copy(c-2) | dma out(c-2)
so the ACT FIFO never parks g~ behind the PSUM-drain copy.

Measured (core 0): 407us (baseline 888us), rel err 1.28e-2 (gate 2e-2).
Engine busy: DVE 373us (96% occupied - the binding engine: 14 min-ops x
208ns x 128 chunks), ACT 331us, PE 348us.  16 nonlinear knot features
per (axis,pixel) are information-theoretically required (16 independent
kink weights per axis); DVE+ACT combined throughput puts the floor of
this design at ~390-410us.
"""

import sys

sys.path.insert(0, "/opt/trn_rl_repo")

import numpy as np

import concourse.bacc as bacc
import concourse.mybir as mybir
import concourse.tile as tile
from concourse.bass_utils import run_bass_kernel_spmd

F32 = mybir.dt.float32
F16 = mybir.dt.float16
EPS = 1e-4
B, C, H, W = 8, 3, 1024, 1024
HW = H * W
NA, K = 8, 16
NK = K + 1
J = 16
NJ = HW // J
CH = 512
NCH = NJ // CH  # 128 chunks
GSH = 8.5

COL_TILE = True
ACT_KNOTS = (1, 2)  # relu(j-g) features have magnitude <= j: keep j small
DVE_KNOTS = tuple(j for j in range(1, NK) if j not in ACT_KNOTS)

_NC_CACHE = {}


def _build_nc():
    nc = bacc.Bacc(None, target_bir_lowering=False, debug=False)
    rawh_t = nc.dram_tensor("rawh", [C, HW], F16, kind="ExternalInput")
    par_t = nc.dram_tensor("par", [128, 2 + len(ACT_KNOTS)], F32, kind="ExternalInput")
    wf2_t = nc.dram_tensor("wf2", [96, 2 * 128], F16, kind="ExternalInput")
    wks_t = nc.dram_tensor("wks", [128, NK * C * J], F16, kind="ExternalInput")
    OPART = 64 + C * J if COL_TILE else C * J
    out0_t = nc.dram_tensor("out0", [OPART, NJ], F16, kind="ExternalOutput")

    Relu = mybir.ActivationFunctionType.Relu
    Ident = mybir.ActivationFunctionType.Identity
    mn = mybir.AluOpType.min

    with tile.TileContext(nc) as tc:
        with (
            tc.tile_pool(name="const", bufs=1) as cpool,
            tc.tile_pool(name="io", bufs=10) as iopool,
            tc.tile_pool(name="gg", bufs=8) as gpool,
            tc.tile_pool(name="rr", bufs=48) as rpool,
            tc.tile_pool(name="ob", bufs=8) as obpool,
            tc.tile_pool(name="pf", bufs=4, space="PSUM") as pfpool,
            tc.tile_pool(name="po", bufs=4, space="PSUM") as popool,
        ):
            pT = cpool.tile([128, 2 + len(ACT_KNOTS)], F32)
            nc.sync.dma_start(out=pT[:], in_=par_t[:])
            wf2 = cpool.tile([96, 2 * 128], F16)
            nc.sync.dma_start(out=wf2[:], in_=wf2_t[:])
            wks = cpool.tile([128, NK * C * J], F16)
            nc.sync.dma_start(out=wks[:], in_=wks_t[:])

            rawh_v = rawh_t.ap().rearrange("c (j n) -> (c j) n", j=J)
            out0_v = out0_t.ap()

            # chunk c pairs: half h = c % 2 of pair p = c // 2
            rhs_tiles = {}
            fps_tiles = {}
            g_tiles = {}
            feat_tiles = {}
            ops_tiles = {}

            def emit_dma_in(p):
                if p >= NCH // 2:
                    return
                n0 = p * 2 * CH
                rhs2 = iopool.tile([96, CH], F16, tag="rhs")
                nc.sync.dma_start(out=rhs2[:48], in_=rawh_v[:, n0 : n0 + CH])
                nc.sync.dma_start(
                    out=rhs2[48:96], in_=rawh_v[:, n0 + CH : n0 + 2 * CH]
                )
                rhs_tiles[p] = rhs2

            def emit_proj(c):
                if c >= NCH:
                    return
                p, h = c // 2, c % 2
                rhs2 = rhs_tiles[p]
                fps = pfpool.tile([128, CH], F32, tag="fps")
                nc.tensor.matmul(
                    fps[0:64], wf2[:, 128 * h : 128 * h + 64], rhs2[:],
                    start=True, stop=True, tile_position=(0, 0),
                )
                nc.tensor.matmul(
                    fps[64:128], wf2[:, 128 * h + 64 : 128 * h + 128], rhs2[:],
                    start=True, stop=True, tile_position=(0, 64),
                )
                fps_tiles[c] = fps

            def emit_g(c):
                if c >= NCH:
                    return
                fps = fps_tiles.pop(c)
                g = gpool.tile([128, CH], F16, tag="g")
                nc.scalar.activation(
                    g[:], fps[:], Ident, bias=pT[:, 1:2], scale=pT[:, 0:1]
                )
                g_tiles[c] = g

            def emit_dve_feats(c):
                if c >= NCH:
                    return
                g = g_tiles[c]
                feats, korder = [g], [0]
                for j in DVE_KNOTS:
                    Rk = rpool.tile([128, CH], F16, tag="R")
                    nc.vector.tensor_scalar(
                        out=Rk[:], in0=g[:], scalar1=float(j) - GSH,
                        scalar2=None, op0=mn,
                    )
                    feats.append(Rk)
                    korder.append(j)
                feat_tiles[c] = (feats, korder)

            def emit_act_feats(c):
                if c >= NCH:
                    return
                g = g_tiles.pop(c)
                feats, korder = feat_tiles[c]
                for i, j in enumerate(ACT_KNOTS):
                    Rk = rpool.tile([128, CH], F16, tag="R")
                    nc.scalar.activation(
                        Rk[:], g[:], Relu, bias=pT[:, 2 + i : 3 + i], scale=-1.0
                    )
                    feats.append(Rk)
                    korder.append(j)

            def emit_knot_mms(c):
                if c >= NCH:
                    return
                feats, korder = feat_tiles.pop(c)
                # interleave so T0/T1 streams alternate: emission order
                # g, dve..., act... -> reorder for PE consumption as produced
                ops = popool.tile([128, CH], F32, tag="ops")
                for ki in range(NK):
                    k = korder[ki]
                    wk = wks[:, k * C * J : (k + 1) * C * J]
                    if COL_TILE:
                        base = 64 * (ki % 2)
                        nc.tensor.matmul(
                            ops[base : base + C * J], wk, feats[ki][:],
                            start=(ki < 2), stop=(ki >= NK - 2),
                            tile_position=(0, base),
                        )
                    else:
                        nc.tensor.matmul(
                            ops[0 : C * J], wk, feats[ki][:],
                            start=(ki == 0), stop=(ki == NK - 1),
                        )
                ops_tiles[c] = ops

            def emit_drain(c):
                if c < 0 or c >= NCH:
                    return
                ops = ops_tiles.pop(c)
                ob = obpool.tile([OPART, CH], F16, tag="ob")
                nc.scalar.copy(ob[:], ops[0:OPART])
                nc.sync.dma_start(
                    out=out0_v[:, c * CH : (c + 1) * CH], in_=ob[:]
                )

            # prologue
            emit_dma_in(0)
            emit_dma_in(1)
            emit_proj(0)
            emit_g(0)
            for c in range(NCH):
                if c % 2 == 0:
                    emit_dma_in(c // 2 + 2)
                emit_proj(c + 1)
                emit_g(c + 1)
                emit_dve_feats(c)
                emit_act_feats(c)
                emit_knot_mms(c)
                emit_drain(c - 2)
            emit_drain(NCH - 2)
            emit_drain(NCH - 1)
    nc.compile()
    return nc


def _host_params(raw, ys, A):
    in_maps = []
    for b in range(B):
        Ab = A[b].astype(np.float32)
        mins = np.minimum(Ab, 0).sum(axis=0)
        maxs = np.maximum(Ab, 0).sum(axis=0)
        pinv = np.linalg.pinv(Ab).astype(np.float32)
        dxf = ((maxs + np.float32(EPS) - mins) / np.float32(NK)).astype(np.float32)
        inv_dx = (np.float32(1.0) / dxf).astype(np.float32)
        off = (-mins * inv_dx).astype(np.float32)
        Y = np.concatenate(
            [mins[:, None], ys[b].astype(np.float32), maxs[:, None]], axis=1
        )
        s = np.diff(Y, axis=1).astype(np.float32)
        coef = np.empty((NA, NK), np.float32)
        coef[:, 0] = s[:, NK - 1]
        coef[:, 1:] = s[:, :-1] - s[:, 1:]

        par = np.zeros((128, 2 + len(ACT_KNOTS)), np.float32)
        par[:, 0] = np.repeat(inv_dx, J)
        par[:, 1] = np.repeat(off, J) - np.float32(GSH)
        for i, j in enumerate(ACT_KNOTS):
            par[:, 2 + i] = np.float32(j) - np.float32(GSH)

        wf = np.zeros((C, J, NA, J), np.float32)
        for j in range(J):
            wf[:, j, :, j] = Ab
        wf = wf.reshape(C * J, NA * J)
        wf2 = np.zeros((96, 2, 128), np.float32)
        wf2[0:48, 0] = wf
        wf2[48:96, 1] = wf
        wf2 = wf2.reshape(96, 2 * 128).astype(np.float16)

        csgn = coef.copy()
        for j in ACT_KNOTS:
            csgn[:, j] = -coef[:, j]
        wks = np.zeros((NA, J, NK, C, J), np.float32)
        for j in range(J):
            wks[:, j, :, :, j] = csgn[:, :, None] * pinv[:, None, :]
        wks = wks.reshape(NA * J, NK * C * J).astype(np.float16)

        b0 = (pinv * Y[:, 0:1]).sum(axis=0).astype(np.float32)
        b0 = b0 + np.float32(GSH) * (pinv * coef[:, 0:1]).sum(axis=0)
        for j in range(1, NK):
            sh = np.float32(j) if j in ACT_KNOTS else np.float32(GSH)
            b0 = b0 + sh * (pinv * coef[:, j : j + 1]).sum(axis=0)

        rh = np.ascontiguousarray(raw[b].reshape(C, HW)).astype(np.float16)
        in_maps.append({"rawh": rh, "par": par, "wf2": wf2, "wks": wks, "_b0": b0})
    return in_maps


def kernel(raw, ys, A):
    raw = np.asarray(raw, np.float32)
    ys = np.asarray(ys, np.float32)
    A = np.asarray(A, np.float32)
    if "nc" not in _NC_CACHE:
        _NC_CACHE["nc"] = _build_nc()
    nc = _NC_CACHE["nc"]
    in_maps = _host_params(raw, ys, A)
    dev_maps = [{k: v for k, v in m.items() if not k.startswith("_")} for m in in_maps]
    res = run_bass_kernel_spmd(nc, dev_maps, core_ids=list(range(B)))
    outs = []
    for b in range(B):
        slab = res.results[b]["out0"].astype(np.float32)
        if COL_TILE:
            o = slab[0 : C * J] + slab[64 : 64 + C * J]
        else:
            o = slab[0 : C * J]
        b0 = in_maps[b]["_b0"]
        o = o.reshape(C, J, NJ) + b0[:, None, None]
        outs.append(o.reshape(C, H, W))
    return np.stack(outs).astype(np.float32)


# revision 31
# speedup vs baseline: 1.2373x; 1.2373x over previous
"""Trainium2 Bass kernel for nn_AxisSimplestSpline (saturation basis, col-tiled
PE, software-pipelined).

Math (per batch b, axis a):  f = A^T raw;  g = (f - mins)/dx in [0,17)
Saturation basis:  est_a(g) = Y0_a + s16_a*g + sum_{j=1..16} w_aj * min(g, j),
w_aj = s_{a,j-1} - s_{a,j}.  Every DVE feature is a single-ALU min ->
4x perf mode (203ns/512).  ACT knots use relu(j - g) = j - min(g,j)
(weight negated, j-term folded into host bias).  g is shifted by -8.5
(gc in [-8.5, 8.5]) to halve feature magnitudes (PE fp16 stream precision
scales with |value|).  All features derive from the SAME fp16-rounded gc
(integer-offset min/relu of fp16 is exact) so rounding errors are
perfectly correlated: total err ~ est(g~)-est(g).
Combine: 17 fp16 matmuls PSUM-accumulated on two concurrent 128x64 array
tiles (even emission -> partitions 0:48, odd -> 64:112); two rhs streams
flow at once (~157ns/512-col MM measured).  Proj matmuls padded to 96
contraction rows to keep tile_size uniform (no PE mode drains).
Software pipeline per 512-col chunk c:
  dma(c+2) | proj(c+1) | ACT g~(c+1) | DVE mins(c) | ACT relus(c)
  | knot MMs(c) | ACT copy(c-2) | dma out(c-2)
so the ACT FIFO never parks g~ behind the PSUM-drain copy.

Measured (core 0): 407us (baseline 888us), rel err 1.28e-2 (gate 2e-2).
Engine busy: DVE 373us (96% occupied - the binding engine: 14 min-ops x
208ns x 128 chunks), ACT 331us, PE 348us.  16 nonlinear knot features
per (axis,pixel) are information-theoretically required (16 independent
kink weights per axis); DVE+ACT combined throughput puts the floor of
this design at ~390-410us.
"""

import sys

sys.path.insert(0, "/opt/trn_rl_repo")

import numpy as np

import concourse.bacc as bacc
import concourse.mybir as mybir
import concourse.tile as tile
from concourse.bass_utils import run_bass_kernel_spmd

F32 = mybir.dt.float32
F16 = mybir.dt.float16
EPS = 1e-4
B, C, H, W = 8, 3, 1024, 1024
HW = H * W
NA, K = 8, 16
NK = K + 1
J = 16
NJ = HW // J
CH = 512
NCH = NJ // CH  # 128 chunks
GSH = 8.5

COL_TILE = True
ACT_KNOTS = (1, 2)  # relu(j-g) features have magnitude <= j: keep j small
DVE_KNOTS = tuple(j for j in range(1, NK) if j not in ACT_KNOTS)

_NC_CACHE = {}


def _build_nc():
    nc = bacc.Bacc(None, target_bir_lowering=False, debug=False)
    rawh_t = nc.dram_tensor("rawh", [C, HW], F16, kind="ExternalInput")
    par_t = nc.dram_tensor("par", [128, 2 + len(ACT_KNOTS)], F32, kind="ExternalInput")
    wf2_t = nc.dram_tensor("wf2", [96, 2 * 128], F16, kind="ExternalInput")
    wks_t = nc.dram_tensor("wks", [128, NK * C * J], F16, kind="ExternalInput")
    OPART = 64 + C * J if COL_TILE else C * J
    out0_t = nc.dram_tensor("out0", [OPART, NJ], F16, kind="ExternalOutput")

    Relu = mybir.ActivationFunctionType.Relu
    Ident = mybir.ActivationFunctionType.Identity
    mn = mybir.AluOpType.min

    with tile.TileContext(nc) as tc:
        with (
            tc.tile_pool(name="const", bufs=1) as cpool,
            tc.tile_pool(name="io", bufs=10) as iopool,
            tc.tile_pool(name="gg", bufs=8) as gpool,
            tc.tile_pool(name="rr", bufs=36) as rpool,
            tc.tile_pool(name="ob", bufs=8) as obpool,
            tc.tile_pool(name="pf", bufs=2, space="PSUM") as pfpool,
            tc.tile_pool(name="po", bufs=2, space="PSUM") as popool,
        ):
            pT = cpool.tile([128, 2 + len(ACT_KNOTS)], F32)
            nc.sync.dma_start(out=pT[:], in_=par_t[:])
            wf2 = cpool.tile([96, 2 * 128], F16)
            nc.sync.dma_start(out=wf2[:], in_=wf2_t[:])
            wks = cpool.tile([128, NK * C * J], F16)
            nc.sync.dma_start(out=wks[:], in_=wks_t[:])

            rawh_v = rawh_t.ap().rearrange("c (j n) -> (c j) n", j=J)
            out0_v = out0_t.ap()

            # super s = 1024 cols = two 512-col PSUM-bank halves
            rhs_tiles = {}
            fps_tiles = {}
            g_tiles = {}
            feat_tiles = {}
            ops_tiles = {}
            NS = NJ // 1024  # 64

            def emit_dma_in(s):
                if s >= NS:
                    return
                n0 = s * 1024
                rhs2 = iopool.tile([96, CH], F16, tag="rhs")
                nc.sync.dma_start(out=rhs2[:48], in_=rawh_v[:, n0 : n0 + CH])
                nc.sync.dma_start(
                    out=rhs2[48:96], in_=rawh_v[:, n0 + CH : n0 + 2 * CH]
                )
                rhs_tiles[s] = rhs2

            def emit_proj(s):
                if s >= NS:
                    return
                rhs2 = rhs_tiles.pop(s)
                fps = pfpool.tile([128, 1024], F32, tag="fps")
                for h in range(2):
                    sl = slice(h * CH, (h + 1) * CH)
                    nc.tensor.matmul(
                        fps[0:64, sl], wf2[:, 128 * h : 128 * h + 64], rhs2[:],
                        start=True, stop=True, tile_position=(0, 0),
                    )
                    nc.tensor.matmul(
                        fps[64:128, sl], wf2[:, 128 * h + 64 : 128 * h + 128],
                        rhs2[:], start=True, stop=True, tile_position=(0, 64),
                    )
                fps_tiles[s] = fps

            def emit_g(s):
                if s >= NS:
                    return
                fps = fps_tiles.pop(s)
                g = gpool.tile([128, 1024], F16, tag="g")
                nc.scalar.activation(
                    g[:], fps[:], Ident, bias=pT[:, 1:2], scale=pT[:, 0:1]
                )
                g_tiles[s] = g

            def emit_dve_feats(s):
                if s >= NS:
                    return
                g = g_tiles[s]
                feats, korder = [g], [0]
                for j in DVE_KNOTS:
                    Rk = rpool.tile([128, 1024], F16, tag="R")
                    nc.vector.tensor_scalar(
                        out=Rk[:], in0=g[:], scalar1=float(j) - GSH,
                        scalar2=None, op0=mn,
                    )
                    feats.append(Rk)
                    korder.append(j)
                feat_tiles[s] = (feats, korder)

            def emit_act_feats(s):
                if s >= NS:
                    return
                g = g_tiles.pop(s)
                feats, korder = feat_tiles[s]
                for i, j in enumerate(ACT_KNOTS):
                    Rk = rpool.tile([128, 1024], F16, tag="R")
                    nc.scalar.activation(
                        Rk[:], g[:], Relu, bias=pT[:, 2 + i : 3 + i], scale=-1.0
                    )
                    feats.append(Rk)
                    korder.append(j)

            def emit_knot_mms(s):
                if s >= NS:
                    return
                feats, korder = feat_tiles.pop(s)
                ops = popool.tile([128, 1024], F32, tag="ops")
                for h in range(2):
                    sl = slice(h * CH, (h + 1) * CH)
                    for ki in range(NK):
                        k = korder[ki]
                        wk = wks[:, k * C * J : (k + 1) * C * J]
                        base = 64 * (ki % 2)
                        nc.tensor.matmul(
                            ops[base : base + C * J, sl], wk, feats[ki][:, sl],
                            start=(ki < 2), stop=(ki >= NK - 2),
                            tile_position=(0, base),
                        )
                ops_tiles[s] = ops

            def emit_drain(s):
                if s < 0 or s >= NS:
                    return
                ops = ops_tiles.pop(s)
                ob = obpool.tile([OPART, 1024], F16, tag="ob")
                nc.scalar.copy(ob[:], ops[0:OPART])
                nc.sync.dma_start(
                    out=out0_v[:, s * 1024 : (s + 1) * 1024], in_=ob[:]
                )

            emit_dma_in(0)
            emit_dma_in(1)
            emit_proj(0)
            emit_g(0)
            for s in range(NS):
                emit_dma_in(s + 2)
                emit_proj(s + 1)
                emit_g(s + 1)
                emit_dve_feats(s)
                emit_act_feats(s)
                emit_knot_mms(s)
                emit_drain(s - 1)
            emit_drain(NS - 1)
    nc.compile()
    return nc


def _host_params(raw, ys, A):
    in_maps = []
    for b in range(B):
        Ab = A[b].astype(np.float32)
        mins = np.minimum(Ab, 0).sum(axis=0)
        maxs = np.maximum(Ab, 0).sum(axis=0)
        pinv = np.linalg.pinv(Ab).astype(np.float32)
        dxf = ((maxs + np.float32(EPS) - mins) / np.float32(NK)).astype(np.float32)
        inv_dx = (np.float32(1.0) / dxf).astype(np.float32)
        off = (-mins * inv_dx).astype(np.float32)
        Y = np.concatenate(
            [mins[:, None], ys[b].astype(np.float32), maxs[:, None]], axis=1
        )
        s = np.diff(Y, axis=1).astype(np.float32)
        coef = np.empty((NA, NK), np.float32)
        coef[:, 0] = s[:, NK - 1]
        coef[:, 1:] = s[:, :-1] - s[:, 1:]

        par = np.zeros((128, 2 + len(ACT_KNOTS)), np.float32)
        par[:, 0] = np.repeat(inv_dx, J)
        par[:, 1] = np.repeat(off, J) - np.float32(GSH)
        for i, j in enumerate(ACT_KNOTS):
            par[:, 2 + i] = np.float32(j) - np.float32(GSH)

        wf = np.zeros((C, J, NA, J), np.float32)
        for j in range(J):
            wf[:, j, :, j] = Ab
        wf = wf.reshape(C * J, NA * J)
        wf2 = np.zeros((96, 2, 128), np.float32)
        wf2[0:48, 0] = wf
        wf2[48:96, 1] = wf
        wf2 = wf2.reshape(96, 2 * 128).astype(np.float16)

        csgn = coef.copy()
        for j in ACT_KNOTS:
            csgn[:, j] = -coef[:, j]
        wks = np.zeros((NA, J, NK, C, J), np.float32)
        for j in range(J):
            wks[:, j, :, :, j] = csgn[:, :, None] * pinv[:, None, :]
        wks = wks.reshape(NA * J, NK * C * J).astype(np.float16)

        b0 = (pinv * Y[:, 0:1]).sum(axis=0).astype(np.float32)
        b0 = b0 + np.float32(GSH) * (pinv * coef[:, 0:1]).sum(axis=0)
        for j in range(1, NK):
            sh = np.float32(j) if j in ACT_KNOTS else np.float32(GSH)
            b0 = b0 + sh * (pinv * coef[:, j : j + 1]).sum(axis=0)

        rh = np.ascontiguousarray(raw[b].reshape(C, HW)).astype(np.float16)
        in_maps.append({"rawh": rh, "par": par, "wf2": wf2, "wks": wks, "_b0": b0})
    return in_maps


def kernel(raw, ys, A):
    raw = np.asarray(raw, np.float32)
    ys = np.asarray(ys, np.float32)
    A = np.asarray(A, np.float32)
    if "nc" not in _NC_CACHE:
        _NC_CACHE["nc"] = _build_nc()
    nc = _NC_CACHE["nc"]
    in_maps = _host_params(raw, ys, A)
    dev_maps = [{k: v for k, v in m.items() if not k.startswith("_")} for m in in_maps]
    res = run_bass_kernel_spmd(nc, dev_maps, core_ids=list(range(B)))
    outs = []
    for b in range(B):
        slab = res.results[b]["out0"].astype(np.float32)
        if COL_TILE:
            o = slab[0 : C * J] + slab[64 : 64 + C * J]
        else:
            o = slab[0 : C * J]
        b0 = in_maps[b]["_b0"]
        o = o.reshape(C, J, NJ) + b0[:, None, None]
        outs.append(o.reshape(C, H, W))
    return np.stack(outs).astype(np.float32)


# revision 32
# speedup vs baseline: 1.2523x; 1.0121x over previous
"""Trainium2 Bass kernel for nn_AxisSimplestSpline (saturation basis, col-tiled
PE, software-pipelined).

Math (per batch b, axis a):  f = A^T raw;  g = (f - mins)/dx in [0,17)
Saturation basis:  est_a(g) = Y0_a + s16_a*g + sum_{j=1..16} w_aj * min(g, j),
w_aj = s_{a,j-1} - s_{a,j}.  Every DVE feature is a single-ALU min ->
4x perf mode (203ns/512).  ACT knots use relu(j - g) = j - min(g,j)
(weight negated, j-term folded into host bias).  g is shifted by -8.5
(gc in [-8.5, 8.5]) to halve feature magnitudes (PE fp16 stream precision
scales with |value|).  All features derive from the SAME fp16-rounded gc
(integer-offset min/relu of fp16 is exact) so rounding errors are
perfectly correlated: total err ~ est(g~)-est(g).
Combine: 17 fp16 matmuls PSUM-accumulated on two concurrent 128x64 array
tiles (even emission -> partitions 0:48, odd -> 64:112); two rhs streams
flow at once (~157ns/512-col MM measured).  Proj matmuls padded to 96
contraction rows to keep tile_size uniform (no PE mode drains).
Software pipeline per 512-col chunk c:
  dma(c+2) | proj(c+1) | ACT g~(c+1) | DVE mins(c) | ACT relus(c)
  | knot MMs(c) | ACT copy(c-2) | dma out(c-2)
so the ACT FIFO never parks g~ behind the PSUM-drain copy.

Measured (core 0): 407us (baseline 888us), rel err 1.28e-2 (gate 2e-2).
Engine busy: DVE 373us (96% occupied - the binding engine: 14 min-ops x
208ns x 128 chunks), ACT 331us, PE 348us.  16 nonlinear knot features
per (axis,pixel) are information-theoretically required (16 independent
kink weights per axis); DVE+ACT combined throughput puts the floor of
this design at ~390-410us.
"""

import sys

sys.path.insert(0, "/opt/trn_rl_repo")

import numpy as np

import concourse.bacc as bacc
import concourse.mybir as mybir
import concourse.tile as tile
from concourse.bass_utils import run_bass_kernel_spmd

F32 = mybir.dt.float32
F16 = mybir.dt.float16
EPS = 1e-4
B, C, H, W = 8, 3, 1024, 1024
HW = H * W
NA, K = 8, 16
NK = K + 1
J = 16
NJ = HW // J
CH = 512
NCH = NJ // CH  # 128 chunks
GSH = 8.5

COL_TILE = True
ACT_KNOTS = (1, 2)  # relu(j-g) features have magnitude <= j: keep j small
DVE_KNOTS = tuple(j for j in range(1, NK) if j not in ACT_KNOTS)

_NC_CACHE = {}


def _build_nc():
    nc = bacc.Bacc(None, target_bir_lowering=False, debug=False)
    rawh_t = nc.dram_tensor("rawh", [C, HW], F16, kind="ExternalInput")
    par_t = nc.dram_tensor("par", [128, 2 + len(ACT_KNOTS)], F32, kind="ExternalInput")
    wf2_t = nc.dram_tensor("wf2", [96, 2 * 128], F16, kind="ExternalInput")
    wks_t = nc.dram_tensor("wks", [128, NK * C * J], F16, kind="ExternalInput")
    OPART = 64 + C * J if COL_TILE else C * J
    out0_t = nc.dram_tensor("out0", [OPART, NJ], F16, kind="ExternalOutput")

    Relu = mybir.ActivationFunctionType.Relu
    Ident = mybir.ActivationFunctionType.Identity
    mn = mybir.AluOpType.min

    with tile.TileContext(nc) as tc:
        with (
            tc.tile_pool(name="const", bufs=1) as cpool,
            tc.tile_pool(name="io", bufs=10) as iopool,
            tc.tile_pool(name="gg", bufs=8) as gpool,
            tc.tile_pool(name="rr", bufs=36) as rpool,
            tc.tile_pool(name="ob", bufs=8) as obpool,
            tc.tile_pool(name="pf", bufs=2, space="PSUM") as pfpool,
            tc.tile_pool(name="po", bufs=2, space="PSUM") as popool,
        ):
            pT = cpool.tile([128, 2 + len(ACT_KNOTS)], F32)
            nc.sync.dma_start(out=pT[:], in_=par_t[:])
            wf2 = cpool.tile([96, 2 * 128], F16)
            nc.sync.dma_start(out=wf2[:], in_=wf2_t[:])
            wks = cpool.tile([128, NK * C * J], F16)
            nc.sync.dma_start(out=wks[:], in_=wks_t[:])

            rawh_v = rawh_t.ap().rearrange("c (j n) -> (c j) n", j=J)
            out0_v = out0_t.ap()

            # super s = 1024 cols = two 512-col PSUM-bank halves
            rhs_tiles = {}
            fps_tiles = {}
            g_tiles = {}
            feat_tiles = {}
            ops_tiles = {}
            NS = NJ // 1024  # 64

            def emit_dma_in(s):
                if s >= NS:
                    return
                n0 = s * 1024
                rhs2 = iopool.tile([96, CH], F16, tag="rhs")
                nc.sync.dma_start(out=rhs2[:48], in_=rawh_v[:, n0 : n0 + CH])
                nc.sync.dma_start(
                    out=rhs2[48:96], in_=rawh_v[:, n0 + CH : n0 + 2 * CH]
                )
                rhs_tiles[s] = rhs2

            def emit_proj(s):
                if s >= NS:
                    return
                rhs2 = rhs_tiles.pop(s)
                fps = pfpool.tile([128, 1024], F32, tag="fps")
                for h in range(2):
                    sl = slice(h * CH, (h + 1) * CH)
                    nc.tensor.matmul(
                        fps[0:64, sl], wf2[:, 128 * h : 128 * h + 64], rhs2[:],
                        start=True, stop=True, tile_position=(0, 0),
                    )
                    nc.tensor.matmul(
                        fps[64:128, sl], wf2[:, 128 * h + 64 : 128 * h + 128],
                        rhs2[:], start=True, stop=True, tile_position=(0, 64),
                    )
                fps_tiles[s] = fps

            def emit_g(s):
                if s >= NS:
                    return
                fps = fps_tiles.pop(s)
                g = gpool.tile([128, 1024], F16, tag="g")
                nc.scalar.activation(
                    g[:], fps[:], Ident, bias=pT[:, 1:2], scale=pT[:, 0:1]
                )
                g_tiles[s] = g

            def emit_dve_feats(s):
                if s >= NS:
                    return
                g = g_tiles[s]
                feats, korder = [g], [0]
                for j in DVE_KNOTS:
                    Rk = rpool.tile([128, 1024], F16, tag="R")
                    nc.vector.tensor_scalar(
                        out=Rk[:], in0=g[:], scalar1=float(j) - GSH,
                        scalar2=None, op0=mn,
                    )
                    feats.append(Rk)
                    korder.append(j)
                feat_tiles[s] = (feats, korder)

            def emit_act_feats(s):
                if s >= NS:
                    return
                g = g_tiles.pop(s)
                feats, korder = feat_tiles[s]
                for i, j in enumerate(ACT_KNOTS):
                    Rk = rpool.tile([128, 1024], F16, tag="R")
                    nc.scalar.activation(
                        Rk[:], g[:], Relu, bias=pT[:, 2 + i : 3 + i], scale=-1.0
                    )
                    feats.append(Rk)
                    korder.append(j)

            def emit_knot_mms(s):
                if s >= NS:
                    return
                feats, korder = feat_tiles.pop(s)
                ops = popool.tile([128, 1024], F32, tag="ops")
                for h in range(2):
                    sl = slice(h * CH, (h + 1) * CH)
                    for ki in range(NK):
                        k = korder[ki]
                        wk = wks[:, k * C * J : (k + 1) * C * J]
                        # flip tile parity on the second half so T0's odd
                        # 9-knot tail pairs with the next half's first MM
                        base = 64 * ((ki + h) % 2)
                        nc.tensor.matmul(
                            ops[base : base + C * J, sl], wk, feats[ki][:, sl],
                            start=(ki < 2), stop=(ki >= NK - 2),
                            tile_position=(0, base),
                        )
                ops_tiles[s] = ops

            def emit_drain(s):
                if s < 0 or s >= NS:
                    return
                ops = ops_tiles.pop(s)
                ob = obpool.tile([OPART, 1024], F16, tag="ob")
                nc.scalar.copy(ob[:], ops[0:OPART])
                nc.sync.dma_start(
                    out=out0_v[:, s * 1024 : (s + 1) * 1024], in_=ob[:]
                )

            emit_dma_in(0)
            emit_dma_in(1)
            emit_proj(0)
            emit_g(0)
            for s in range(NS):
                emit_dma_in(s + 2)
                emit_proj(s + 1)
                emit_g(s + 1)
                emit_dve_feats(s)
                emit_act_feats(s)
                emit_knot_mms(s)
                emit_drain(s - 1)
            emit_drain(NS - 1)
    nc.compile()
    return nc


def _host_params(raw, ys, A):
    in_maps = []
    for b in range(B):
        Ab = A[b].astype(np.float32)
        mins = np.minimum(Ab, 0).sum(axis=0)
        maxs = np.maximum(Ab, 0).sum(axis=0)
        pinv = np.linalg.pinv(Ab).astype(np.float32)
        dxf = ((maxs + np.float32(EPS) - mins) / np.float32(NK)).astype(np.float32)
        inv_dx = (np.float32(1.0) / dxf).astype(np.float32)
        off = (-mins * inv_dx).astype(np.float32)
        Y = np.concatenate(
            [mins[:, None], ys[b].astype(np.float32), maxs[:, None]], axis=1
        )
        s = np.diff(Y, axis=1).astype(np.float32)
        coef = np.empty((NA, NK), np.float32)
        coef[:, 0] = s[:, NK - 1]
        coef[:, 1:] = s[:, :-1] - s[:, 1:]

        par = np.zeros((128, 2 + len(ACT_KNOTS)), np.float32)
        par[:, 0] = np.repeat(inv_dx, J)
        par[:, 1] = np.repeat(off, J) - np.float32(GSH)
        for i, j in enumerate(ACT_KNOTS):
            par[:, 2 + i] = np.float32(j) - np.float32(GSH)

        wf = np.zeros((C, J, NA, J), np.float32)
        for j in range(J):
            wf[:, j, :, j] = Ab
        wf = wf.reshape(C * J, NA * J)
        wf2 = np.zeros((96, 2, 128), np.float32)
        wf2[0:48, 0] = wf
        wf2[48:96, 1] = wf
        wf2 = wf2.reshape(96, 2 * 128).astype(np.float16)

        csgn = coef.copy()
        for j in ACT_KNOTS:
            csgn[:, j] = -coef[:, j]
        wks = np.zeros((NA, J, NK, C, J), np.float32)
        for j in range(J):
            wks[:, j, :, :, j] = csgn[:, :, None] * pinv[:, None, :]
        wks = wks.reshape(NA * J, NK * C * J).astype(np.float16)

        b0 = (pinv * Y[:, 0:1]).sum(axis=0).astype(np.float32)
        b0 = b0 + np.float32(GSH) * (pinv * coef[:, 0:1]).sum(axis=0)
        for j in range(1, NK):
            sh = np.float32(j) if j in ACT_KNOTS else np.float32(GSH)
            b0 = b0 + sh * (pinv * coef[:, j : j + 1]).sum(axis=0)

        rh = np.ascontiguousarray(raw[b].reshape(C, HW)).astype(np.float16)
        in_maps.append({"rawh": rh, "par": par, "wf2": wf2, "wks": wks, "_b0": b0})
    return in_maps


def kernel(raw, ys, A):
    raw = np.asarray(raw, np.float32)
    ys = np.asarray(ys, np.float32)
    A = np.asarray(A, np.float32)
    if "nc" not in _NC_CACHE:
        _NC_CACHE["nc"] = _build_nc()
    nc = _NC_CACHE["nc"]
    in_maps = _host_params(raw, ys, A)
    dev_maps = [{k: v for k, v in m.items() if not k.startswith("_")} for m in in_maps]
    res = run_bass_kernel_spmd(nc, dev_maps, core_ids=list(range(B)))
    outs = []
    for b in range(B):
        slab = res.results[b]["out0"].astype(np.float32)
        if COL_TILE:
            o = slab[0 : C * J] + slab[64 : 64 + C * J]
        else:
            o = slab[0 : C * J]
        b0 = in_maps[b]["_b0"]
        o = o.reshape(C, J, NJ) + b0[:, None, None]
        outs.append(o.reshape(C, H, W))
    return np.stack(outs).astype(np.float32)


# revision 35
# speedup vs baseline: 1.2529x; 1.0005x over previous
"""Trainium2 Bass kernel for nn_AxisSimplestSpline (saturation basis, col-tiled
PE, software-pipelined).

Math (per batch b, axis a):  f = A^T raw;  g = (f - mins)/dx in [0,17)
Saturation basis:  est_a(g) = Y0_a + s16_a*g + sum_{j=1..16} w_aj * min(g, j),
w_aj = s_{a,j-1} - s_{a,j}.  Every DVE feature is a single-ALU min ->
4x perf mode (203ns/512).  ACT knots use relu(j - g) = j - min(g,j)
(weight negated, j-term folded into host bias).  g is shifted by -8.5
(gc in [-8.5, 8.5]) to halve feature magnitudes (PE fp16 stream precision
scales with |value|).  All features derive from the SAME fp16-rounded gc
(integer-offset min/relu of fp16 is exact) so rounding errors are
perfectly correlated: total err ~ est(g~)-est(g).
Combine: 17 fp16 matmuls PSUM-accumulated on two concurrent 128x64 array
tiles (even emission -> partitions 0:48, odd -> 64:112); two rhs streams
flow at once (~157ns/512-col MM measured).  Proj matmuls padded to 96
contraction rows to keep tile_size uniform (no PE mode drains).
Software pipeline per 1024-col super s:
  dma(s+2) | proj(s+1) | ACT g~(s+1) | DVE mins(s) | ACT relus(s)
  | knot MMs(s) | ACT copy(s-1) | dma out(s-1)
so the ACT FIFO never parks g~ behind the PSUM-drain copy.

Measured (core 0): 326us (baseline 888us, 2.7x), rel err 1.28e-2
(gate 2e-2).  Engine busy at 330us total: PE 312us (99.9% occupied,
binding), DVE 303us (14 min-ops x 338ns x 64 supers), ACT 279us.
All ops at 1024-col super granularity (PSUM tiles span 2 banks; MMs
address 512-col halves); T0/T1 tile parity flips on the second half so
the 9-knot tail of one half pairs with the next half's first MM.
16 nonlinear knot features per (axis,pixel) are information-required
(16 independent kink weights/axis); PE floor ~260us, DVE ~300us.
"""

import sys

sys.path.insert(0, "/opt/trn_rl_repo")

import numpy as np

import concourse.bacc as bacc
import concourse.mybir as mybir
import concourse.tile as tile
from concourse.bass_utils import run_bass_kernel_spmd

F32 = mybir.dt.float32
F16 = mybir.dt.float16
EPS = 1e-4
B, C, H, W = 8, 3, 1024, 1024
HW = H * W
NA, K = 8, 16
NK = K + 1
J = 16
NJ = HW // J
CH = 512
NCH = NJ // CH  # 128 chunks
GSH = 8.5

COL_TILE = True
ACT_KNOTS = (1, 2)  # relu(j-g) features have magnitude <= j: keep j small
DVE_KNOTS = tuple(j for j in range(1, NK) if j not in ACT_KNOTS)

_NC_CACHE = {}


def _build_nc():
    nc = bacc.Bacc(None, target_bir_lowering=False, debug=False)
    rawh_t = nc.dram_tensor("rawh", [C, HW], F16, kind="ExternalInput")
    par_t = nc.dram_tensor("par", [128, 2 + len(ACT_KNOTS)], F32, kind="ExternalInput")
    wf2_t = nc.dram_tensor("wf2", [96, 2 * 128], F16, kind="ExternalInput")
    wks_t = nc.dram_tensor("wks", [128, NK * C * J], F16, kind="ExternalInput")
    OPART = 64 + C * J if COL_TILE else C * J
    out0_t = nc.dram_tensor("out0", [OPART, NJ], F16, kind="ExternalOutput")

    Relu = mybir.ActivationFunctionType.Relu
    Ident = mybir.ActivationFunctionType.Identity
    mn = mybir.AluOpType.min

    with tile.TileContext(nc) as tc:
        with (
            tc.tile_pool(name="const", bufs=1) as cpool,
            tc.tile_pool(name="io", bufs=10) as iopool,
            tc.tile_pool(name="gg", bufs=8) as gpool,
            tc.tile_pool(name="rr", bufs=36) as rpool,
            tc.tile_pool(name="ob", bufs=8) as obpool,
            tc.tile_pool(name="pf", bufs=2, space="PSUM") as pfpool,
            tc.tile_pool(name="po", bufs=2, space="PSUM") as popool,
        ):
            pT = cpool.tile([128, 2 + len(ACT_KNOTS)], F32)
            nc.sync.dma_start(out=pT[:], in_=par_t[:])
            wf2 = cpool.tile([96, 2 * 128], F16)
            nc.sync.dma_start(out=wf2[:], in_=wf2_t[:])
            wks = cpool.tile([128, NK * C * J], F16)
            nc.sync.dma_start(out=wks[:], in_=wks_t[:])

            rawh_v = rawh_t.ap().rearrange("c (j n) -> (c j) n", j=J)
            out0_v = out0_t.ap()

            # super s = 1024 cols = two 512-col PSUM-bank halves
            rhs_tiles = {}
            fps_tiles = {}
            g_tiles = {}
            feat_tiles = {}
            ops_tiles = {}
            NS = NJ // 1024  # 64

            def emit_dma_in(s):
                if s >= NS:
                    return
                n0 = s * 1024
                rhs2 = iopool.tile([96, CH], F16, tag="rhs")
                nc.sync.dma_start(out=rhs2[:48], in_=rawh_v[:, n0 : n0 + CH])
                nc.sync.dma_start(
                    out=rhs2[48:96], in_=rawh_v[:, n0 + CH : n0 + 2 * CH]
                )
                rhs_tiles[s] = rhs2

            def emit_proj(s):
                if s >= NS:
                    return
                rhs2 = rhs_tiles.pop(s)
                fps = pfpool.tile([128, 1024], F32, tag="fps")
                for h in range(2):
                    sl = slice(h * CH, (h + 1) * CH)
                    nc.tensor.matmul(
                        fps[0:64, sl], wf2[:, 128 * h : 128 * h + 64], rhs2[:],
                        start=True, stop=True, tile_position=(0, 0),
                    )
                    nc.tensor.matmul(
                        fps[64:128, sl], wf2[:, 128 * h + 64 : 128 * h + 128],
                        rhs2[:], start=True, stop=True, tile_position=(0, 64),
                    )
                fps_tiles[s] = fps

            def emit_g(s):
                if s >= NS:
                    return
                fps = fps_tiles.pop(s)
                g = gpool.tile([128, 1024], F16, tag="g")
                nc.scalar.activation(
                    g[:], fps[:], Ident, bias=pT[:, 1:2], scale=pT[:, 0:1]
                )
                g_tiles[s] = g

            def emit_dve_feats(s):
                if s >= NS:
                    return
                g = g_tiles[s]
                feats, korder = [g], [0]
                for j in DVE_KNOTS:
                    Rk = rpool.tile([128, 1024], F16, tag="R")
                    nc.vector.tensor_scalar(
                        out=Rk[:], in0=g[:], scalar1=float(j) - GSH,
                        scalar2=None, op0=mn,
                    )
                    feats.append(Rk)
                    korder.append(j)
                feat_tiles[s] = (feats, korder)

            def emit_act_feats(s):
                if s >= NS:
                    return
                g = g_tiles.pop(s)
                feats, korder = feat_tiles[s]
                for i, j in enumerate(ACT_KNOTS):
                    Rk = rpool.tile([128, 1024], F16, tag="R")
                    nc.scalar.activation(
                        Rk[:], g[:], Relu, bias=pT[:, 2 + i : 3 + i], scale=-1.0
                    )
                    feats.append(Rk)
                    korder.append(j)

            def emit_knot_mms(s):
                if s >= NS:
                    return
                feats, korder = feat_tiles.pop(s)
                ops = popool.tile([128, 1024], F32, tag="ops")
                for h in range(2):
                    sl = slice(h * CH, (h + 1) * CH)
                    for ki in range(NK):
                        k = korder[ki]
                        wk = wks[:, k * C * J : (k + 1) * C * J]
                        # flip tile parity on the second half so T0's odd
                        # 9-knot tail pairs with the next half's first MM
                        base = 64 * ((ki + h) % 2)
                        nc.tensor.matmul(
                            ops[base : base + C * J, sl], wk, feats[ki][:, sl],
                            start=(ki < 2), stop=(ki >= NK - 2),
                            tile_position=(0, base),
                        )
                ops_tiles[s] = ops

            def emit_drain(s):
                if s < 0 or s >= NS:
                    return
                ops = ops_tiles.pop(s)
                ob = obpool.tile([OPART, 1024], F16, tag="ob")
                nc.scalar.copy(ob[:], ops[0:OPART])
                nc.sync.dma_start(
                    out=out0_v[:, s * 1024 : (s + 1) * 1024], in_=ob[:]
                )

            emit_dma_in(0)
            emit_dma_in(1)
            emit_proj(0)
            emit_g(0)
            for s in range(NS):
                emit_dma_in(s + 2)
                emit_proj(s + 1)
                emit_g(s + 1)
                emit_dve_feats(s)
                emit_act_feats(s)
                emit_knot_mms(s)
                emit_drain(s - 1)
            emit_drain(NS - 1)
    nc.compile()
    return nc


def _host_params(raw, ys, A):
    in_maps = []
    for b in range(B):
        Ab = A[b].astype(np.float32)
        mins = np.minimum(Ab, 0).sum(axis=0)
        maxs = np.maximum(Ab, 0).sum(axis=0)
        pinv = np.linalg.pinv(Ab).astype(np.float32)
        dxf = ((maxs + np.float32(EPS) - mins) / np.float32(NK)).astype(np.float32)
        inv_dx = (np.float32(1.0) / dxf).astype(np.float32)
        off = (-mins * inv_dx).astype(np.float32)
        Y = np.concatenate(
            [mins[:, None], ys[b].astype(np.float32), maxs[:, None]], axis=1
        )
        s = np.diff(Y, axis=1).astype(np.float32)
        coef = np.empty((NA, NK), np.float32)
        coef[:, 0] = s[:, NK - 1]
        coef[:, 1:] = s[:, :-1] - s[:, 1:]

        par = np.zeros((128, 2 + len(ACT_KNOTS)), np.float32)
        par[:, 0] = np.repeat(inv_dx, J)
        par[:, 1] = np.repeat(off, J) - np.float32(GSH)
        for i, j in enumerate(ACT_KNOTS):
            par[:, 2 + i] = np.float32(j) - np.float32(GSH)

        wf = np.zeros((C, J, NA, J), np.float32)
        for j in range(J):
            wf[:, j, :, j] = Ab
        wf = wf.reshape(C * J, NA * J)
        wf2 = np.zeros((96, 2, 128), np.float32)
        wf2[0:48, 0] = wf
        wf2[48:96, 1] = wf
        wf2 = wf2.reshape(96, 2 * 128).astype(np.float16)

        csgn = coef.copy()
        for j in ACT_KNOTS:
            csgn[:, j] = -coef[:, j]
        wks = np.zeros((NA, J, NK, C, J), np.float32)
        for j in range(J):
            wks[:, j, :, :, j] = csgn[:, :, None] * pinv[:, None, :]
        wks = wks.reshape(NA * J, NK * C * J).astype(np.float16)

        b0 = (pinv * Y[:, 0:1]).sum(axis=0).astype(np.float32)
        b0 = b0 + np.float32(GSH) * (pinv * coef[:, 0:1]).sum(axis=0)
        for j in range(1, NK):
            sh = np.float32(j) if j in ACT_KNOTS else np.float32(GSH)
            b0 = b0 + sh * (pinv * coef[:, j : j + 1]).sum(axis=0)

        rh = np.ascontiguousarray(raw[b].reshape(C, HW)).astype(np.float16)
        in_maps.append({"rawh": rh, "par": par, "wf2": wf2, "wks": wks, "_b0": b0})
    return in_maps


def kernel(raw, ys, A):
    raw = np.asarray(raw, np.float32)
    ys = np.asarray(ys, np.float32)
    A = np.asarray(A, np.float32)
    if "nc" not in _NC_CACHE:
        _NC_CACHE["nc"] = _build_nc()
    nc = _NC_CACHE["nc"]
    in_maps = _host_params(raw, ys, A)
    dev_maps = [{k: v for k, v in m.items() if not k.startswith("_")} for m in in_maps]
    res = run_bass_kernel_spmd(nc, dev_maps, core_ids=list(range(B)))
    outs = []
    for b in range(B):
        slab = res.results[b]["out0"].astype(np.float32)
        if COL_TILE:
            o = slab[0 : C * J] + slab[64 : 64 + C * J]
        else:
            o = slab[0 : C * J]
        b0 = in_maps[b]["_b0"]
        o = o.reshape(C, J, NJ) + b0[:, None, None]
        outs.append(o.reshape(C, H, W))
    return np.stack(outs).astype(np.float32)


# revision 36
# speedup vs baseline: 1.2644x; 1.0092x over previous
"""Trainium2 Bass kernel for nn_AxisSimplestSpline (saturation basis, col-tiled
PE, software-pipelined).

Math (per batch b, axis a):  f = A^T raw;  g = (f - mins)/dx in [0,17)
Saturation basis:  est_a(g) = Y0_a + s16_a*g + sum_{j=1..16} w_aj * min(g, j),
w_aj = s_{a,j-1} - s_{a,j}.  Every DVE feature is a single-ALU min ->
4x perf mode (203ns/512).  ACT knots use relu(j - g) = j - min(g,j)
(weight negated, j-term folded into host bias).  g is shifted by -8.5
(gc in [-8.5, 8.5]) to halve feature magnitudes (PE fp16 stream precision
scales with |value|).  All features derive from the SAME fp16-rounded gc
(integer-offset min/relu of fp16 is exact) so rounding errors are
perfectly correlated: total err ~ est(g~)-est(g).
Combine: 17 fp16 matmuls PSUM-accumulated on two concurrent 128x64 array
tiles (even emission -> partitions 0:48, odd -> 64:112); two rhs streams
flow at once (~157ns/512-col MM measured).  Proj matmuls padded to 96
contraction rows to keep tile_size uniform (no PE mode drains).
Software pipeline per 1024-col super s:
  dma(s+2) | proj(s+1) | ACT g~(s+1) | DVE mins(s) | ACT relus(s)
  | knot MMs(s) | ACT copy(s-1) | dma out(s-1)
so the ACT FIFO never parks g~ behind the PSUM-drain copy.

Measured (core 0): 326us (baseline 888us, 2.7x), rel err 1.28e-2
(gate 2e-2).  Engine busy at 330us total: PE 312us (99.9% occupied,
binding), DVE 303us (14 min-ops x 338ns x 64 supers), ACT 279us.
All ops at 1024-col super granularity (PSUM tiles span 2 banks; MMs
address 512-col halves); T0/T1 tile parity flips on the second half so
the 9-knot tail of one half pairs with the next half's first MM.
16 nonlinear knot features per (axis,pixel) are information-required
(16 independent kink weights/axis); PE floor ~260us, DVE ~300us.
"""

import sys

sys.path.insert(0, "/opt/trn_rl_repo")

import numpy as np

import concourse.bacc as bacc
import concourse.mybir as mybir
import concourse.tile as tile
from concourse.bass_utils import run_bass_kernel_spmd

F32 = mybir.dt.float32
F16 = mybir.dt.float16
EPS = 1e-4
B, C, H, W = 8, 3, 1024, 1024
HW = H * W
NA, K = 8, 16
NK = K + 1
J = 16
NJ = HW // J
CH = 512
NCH = NJ // CH  # 128 chunks
GSH = 8.5

COL_TILE = True
ACT_KNOTS = (1, 2)  # relu(j-g) features have magnitude <= j: keep j small
DVE_KNOTS = tuple(j for j in range(1, NK) if j not in ACT_KNOTS)

_NC_CACHE = {}


def _build_nc():
    nc = bacc.Bacc(None, target_bir_lowering=False, debug=False)
    rawh_t = nc.dram_tensor("rawh", [C, HW], F16, kind="ExternalInput")
    par_t = nc.dram_tensor("par", [128, 2 + len(ACT_KNOTS)], F32, kind="ExternalInput")
    wf2_t = nc.dram_tensor("wf2", [96, 2 * 128], F16, kind="ExternalInput")
    wks_t = nc.dram_tensor("wks", [128, NK * C * J], F16, kind="ExternalInput")
    OPART = 64 + C * J if COL_TILE else C * J
    out0_t = nc.dram_tensor("out0", [OPART, NJ], F16, kind="ExternalOutput")

    Relu = mybir.ActivationFunctionType.Relu
    Ident = mybir.ActivationFunctionType.Identity
    mn = mybir.AluOpType.min

    with tile.TileContext(nc) as tc:
        with (
            tc.tile_pool(name="const", bufs=1) as cpool,
            tc.tile_pool(name="io", bufs=10) as iopool,
            tc.tile_pool(name="gg", bufs=8) as gpool,
            tc.tile_pool(name="rr", bufs=36) as rpool,
            tc.tile_pool(name="ob", bufs=8) as obpool,
            tc.tile_pool(name="pf", bufs=2, space="PSUM") as pfpool,
            tc.tile_pool(name="po", bufs=2, space="PSUM") as popool,
        ):
            pT = cpool.tile([128, 2 + len(ACT_KNOTS)], F32)
            nc.sync.dma_start(out=pT[:], in_=par_t[:])
            wf2 = cpool.tile([96, 2 * 128], F16)
            nc.sync.dma_start(out=wf2[:], in_=wf2_t[:])
            wks = cpool.tile([128, NK * C * J], F16)
            nc.sync.dma_start(out=wks[:], in_=wks_t[:])

            rawh_v = rawh_t.ap().rearrange("c (j n) -> (c j) n", j=J)
            out0_v = out0_t.ap()

            # super s = 1024 cols = two 512-col PSUM-bank halves
            rhs_tiles = {}
            fps_tiles = {}
            g_tiles = {}
            feat_tiles = {}
            ops_tiles = {}
            NS = NJ // 1024  # 64

            def emit_dma_in(s):
                if s >= NS:
                    return
                n0 = s * 1024
                rhs2 = iopool.tile([96, CH], F16, tag="rhs")
                nc.sync.dma_start(out=rhs2[:48], in_=rawh_v[:, n0 : n0 + CH])
                nc.sync.dma_start(
                    out=rhs2[48:96], in_=rawh_v[:, n0 + CH : n0 + 2 * CH]
                )
                rhs_tiles[s] = rhs2

            def emit_proj(s):
                if s >= NS:
                    return
                rhs2 = rhs_tiles.pop(s)
                fps = pfpool.tile([128, 1024], F32, tag="fps")
                for h in range(2):
                    sl = slice(h * CH, (h + 1) * CH)
                    nc.tensor.matmul(
                        fps[0:64, sl], wf2[:, 128 * h : 128 * h + 64], rhs2[:],
                        start=True, stop=True, tile_position=(0, 0),
                    )
                    nc.tensor.matmul(
                        fps[64:128, sl], wf2[:, 128 * h + 64 : 128 * h + 128],
                        rhs2[:], start=True, stop=True, tile_position=(0, 64),
                    )
                fps_tiles[s] = fps

            def emit_g(s):
                if s >= NS:
                    return
                fps = fps_tiles.pop(s)
                g = gpool.tile([128, 1024], F16, tag="g")
                nc.scalar.activation(
                    g[:], fps[:], Ident, bias=pT[:, 1:2], scale=pT[:, 0:1]
                )
                g_tiles[s] = g

            def emit_dve_feats(s):
                if s >= NS:
                    return
                g = g_tiles[s]
                feats, korder = [g], [0]
                for j in DVE_KNOTS:
                    Rk = rpool.tile([128, 1024], F16, tag="R")
                    nc.vector.tensor_scalar(
                        out=Rk[:], in0=g[:], scalar1=float(j) - GSH,
                        scalar2=None, op0=mn,
                    )
                    feats.append(Rk)
                    korder.append(j)
                feat_tiles[s] = (feats, korder)

            def emit_act_feats(s):
                if s >= NS:
                    return
                g = g_tiles.pop(s)
                feats, korder = feat_tiles[s]
                for i, j in enumerate(ACT_KNOTS):
                    Rk = rpool.tile([128, 1024], F16, tag="R")
                    nc.scalar.activation(
                        Rk[:], g[:], Relu, bias=pT[:, 2 + i : 3 + i], scale=-1.0
                    )
                    feats.append(Rk)
                    korder.append(j)

            def emit_knot_mms(s):
                if s >= NS:
                    return
                feats, korder = feat_tiles.pop(s)
                ops = popool.tile([128, 1024], F32, tag="ops")
                # knot-major: both 512-col halves of a knot run back-to-back
                # on the same tile (one weight load per knot); per-super
                # parity flip pairs the odd 17th knot across supers
                for ki in range(NK):
                    k = korder[ki]
                    wk = wks[:, k * C * J : (k + 1) * C * J]
                    base = 64 * ((ki + s) % 2)
                    for h in range(2):
                        sl = slice(h * CH, (h + 1) * CH)
                        nc.tensor.matmul(
                            ops[base : base + C * J, sl], wk, feats[ki][:, sl],
                            start=(ki < 2), stop=(ki >= NK - 2),
                            tile_position=(0, base),
                        )
                ops_tiles[s] = ops

            def emit_drain(s):
                if s < 0 or s >= NS:
                    return
                ops = ops_tiles.pop(s)
                ob = obpool.tile([OPART, 1024], F16, tag="ob")
                nc.scalar.copy(ob[:], ops[0:OPART])
                nc.sync.dma_start(
                    out=out0_v[:, s * 1024 : (s + 1) * 1024], in_=ob[:]
                )

            emit_dma_in(0)
            emit_dma_in(1)
            emit_proj(0)
            emit_g(0)
            for s in range(NS):
                emit_dma_in(s + 2)
                emit_proj(s + 1)
                emit_g(s + 1)
                emit_dve_feats(s)
                emit_act_feats(s)
                emit_knot_mms(s)
                emit_drain(s - 1)
            emit_drain(NS - 1)
    nc.compile()
    return nc


def _host_params(raw, ys, A):
    in_maps = []
    for b in range(B):
        Ab = A[b].astype(np.float32)
        mins = np.minimum(Ab, 0).sum(axis=0)
        maxs = np.maximum(Ab, 0).sum(axis=0)
        pinv = np.linalg.pinv(Ab).astype(np.float32)
        dxf = ((maxs + np.float32(EPS) - mins) / np.float32(NK)).astype(np.float32)
        inv_dx = (np.float32(1.0) / dxf).astype(np.float32)
        off = (-mins * inv_dx).astype(np.float32)
        Y = np.concatenate(
            [mins[:, None], ys[b].astype(np.float32), maxs[:, None]], axis=1
        )
        s = np.diff(Y, axis=1).astype(np.float32)
        coef = np.empty((NA, NK), np.float32)
        coef[:, 0] = s[:, NK - 1]
        coef[:, 1:] = s[:, :-1] - s[:, 1:]

        par = np.zeros((128, 2 + len(ACT_KNOTS)), np.float32)
        par[:, 0] = np.repeat(inv_dx, J)
        par[:, 1] = np.repeat(off, J) - np.float32(GSH)
        for i, j in enumerate(ACT_KNOTS):
            par[:, 2 + i] = np.float32(j) - np.float32(GSH)

        wf = np.zeros((C, J, NA, J), np.float32)
        for j in range(J):
            wf[:, j, :, j] = Ab
        wf = wf.reshape(C * J, NA * J)
        wf2 = np.zeros((96, 2, 128), np.float32)
        wf2[0:48, 0] = wf
        wf2[48:96, 1] = wf
        wf2 = wf2.reshape(96, 2 * 128).astype(np.float16)

        csgn = coef.copy()
        for j in ACT_KNOTS:
            csgn[:, j] = -coef[:, j]
        wks = np.zeros((NA, J, NK, C, J), np.float32)
        for j in range(J):
            wks[:, j, :, :, j] = csgn[:, :, None] * pinv[:, None, :]
        wks = wks.reshape(NA * J, NK * C * J).astype(np.float16)

        b0 = (pinv * Y[:, 0:1]).sum(axis=0).astype(np.float32)
        b0 = b0 + np.float32(GSH) * (pinv * coef[:, 0:1]).sum(axis=0)
        for j in range(1, NK):
            sh = np.float32(j) if j in ACT_KNOTS else np.float32(GSH)
            b0 = b0 + sh * (pinv * coef[:, j : j + 1]).sum(axis=0)

        rh = np.ascontiguousarray(raw[b].reshape(C, HW)).astype(np.float16)
        in_maps.append({"rawh": rh, "par": par, "wf2": wf2, "wks": wks, "_b0": b0})
    return in_maps


def kernel(raw, ys, A):
    raw = np.asarray(raw, np.float32)
    ys = np.asarray(ys, np.float32)
    A = np.asarray(A, np.float32)
    if "nc" not in _NC_CACHE:
        _NC_CACHE["nc"] = _build_nc()
    nc = _NC_CACHE["nc"]
    in_maps = _host_params(raw, ys, A)
    dev_maps = [{k: v for k, v in m.items() if not k.startswith("_")} for m in in_maps]
    res = run_bass_kernel_spmd(nc, dev_maps, core_ids=list(range(B)))
    outs = []
    for b in range(B):
        slab = res.results[b]["out0"].astype(np.float32)
        if COL_TILE:
            o = slab[0 : C * J] + slab[64 : 64 + C * J]
        else:
            o = slab[0 : C * J]
        b0 = in_maps[b]["_b0"]
        o = o.reshape(C, J, NJ) + b0[:, None, None]
        outs.append(o.reshape(C, H, W))
    return np.stack(outs).astype(np.float32)


# revision 37
# speedup vs baseline: 1.2654x; 1.0008x over previous
"""Trainium2 Bass kernel for nn_AxisSimplestSpline (saturation basis, col-tiled
PE, software-pipelined).

Math (per batch b, axis a):  f = A^T raw;  g = (f - mins)/dx in [0,17)
Saturation basis:  est_a(g) = Y0_a + s16_a*g + sum_{j=1..16} w_aj * min(g, j),
w_aj = s_{a,j-1} - s_{a,j}.  Every DVE feature is a single-ALU min ->
4x perf mode (203ns/512).  ACT knots use relu(j - g) = j - min(g,j)
(weight negated, j-term folded into host bias).  g is shifted by -8.5
(gc in [-8.5, 8.5]) to halve feature magnitudes (PE fp16 stream precision
scales with |value|).  All features derive from the SAME fp16-rounded gc
(integer-offset min/relu of fp16 is exact) so rounding errors are
perfectly correlated: total err ~ est(g~)-est(g).
Combine: 17 fp16 matmuls PSUM-accumulated on two concurrent 128x64 array
tiles (even emission -> partitions 0:48, odd -> 64:112); two rhs streams
flow at once (~157ns/512-col MM measured).  Proj matmuls padded to 96
contraction rows to keep tile_size uniform (no PE mode drains).
Software pipeline per 1024-col super s:
  dma(s+2) | proj(s+1) | ACT g~(s+1) | DVE mins(s) | ACT relus(s)
  | knot MMs(s) | ACT copy(s-1) | dma out(s-1)
so the ACT FIFO never parks g~ behind the PSUM-drain copy.

Measured (core 0): 322.7us (baseline 888us, 2.75x), rel err 1.28e-2
(gate 2e-2).  Engine busy: PE ~312us (99.9% occupied, binding), DVE
303us (14 min-ops x 338ns x 64 supers), ACT 279us.  All ops at 1024-col
super granularity (PSUM tiles span 2 banks; MMs address 512-col halves
- the ISA rejects 1024-col MM free dims).  Knot MMs are knot-major
(both halves back-to-back on one tile -> one weight load per knot) with
per-super T0/T1 parity flip to pair the odd 17th knot across supers.
16 nonlinear knot features per (axis,pixel) are information-required
(16 independent kink weights/axis); PE floor ~260us, DVE ~300us.
"""

import sys

sys.path.insert(0, "/opt/trn_rl_repo")

import numpy as np

import concourse.bacc as bacc
import concourse.mybir as mybir
import concourse.tile as tile
from concourse.bass_utils import run_bass_kernel_spmd

F32 = mybir.dt.float32
F16 = mybir.dt.float16
EPS = 1e-4
B, C, H, W = 8, 3, 1024, 1024
HW = H * W
NA, K = 8, 16
NK = K + 1
J = 16
NJ = HW // J
CH = 512
NCH = NJ // CH  # 128 chunks
GSH = 8.5

COL_TILE = True
ACT_KNOTS = (1, 2)  # relu(j-g) features have magnitude <= j: keep j small
DVE_KNOTS = tuple(j for j in range(1, NK) if j not in ACT_KNOTS)

_NC_CACHE = {}


def _build_nc():
    nc = bacc.Bacc(None, target_bir_lowering=False, debug=False)
    rawh_t = nc.dram_tensor("rawh", [C, HW], F16, kind="ExternalInput")
    par_t = nc.dram_tensor("par", [128, 2 + len(ACT_KNOTS)], F32, kind="ExternalInput")
    wf2_t = nc.dram_tensor("wf2", [96, 2 * 128], F16, kind="ExternalInput")
    wks_t = nc.dram_tensor("wks", [128, NK * C * J], F16, kind="ExternalInput")
    OPART = 64 + C * J if COL_TILE else C * J
    out0_t = nc.dram_tensor("out0", [OPART, NJ], F16, kind="ExternalOutput")

    Relu = mybir.ActivationFunctionType.Relu
    Ident = mybir.ActivationFunctionType.Identity
    mn = mybir.AluOpType.min

    with tile.TileContext(nc) as tc:
        with (
            tc.tile_pool(name="const", bufs=1) as cpool,
            tc.tile_pool(name="io", bufs=10) as iopool,
            tc.tile_pool(name="gg", bufs=8) as gpool,
            tc.tile_pool(name="rr", bufs=36) as rpool,
            tc.tile_pool(name="ob", bufs=8) as obpool,
            tc.tile_pool(name="pf", bufs=2, space="PSUM") as pfpool,
            tc.tile_pool(name="po", bufs=2, space="PSUM") as popool,
        ):
            pT = cpool.tile([128, 2 + len(ACT_KNOTS)], F32)
            nc.sync.dma_start(out=pT[:], in_=par_t[:])
            wf2 = cpool.tile([96, 2 * 128], F16)
            nc.sync.dma_start(out=wf2[:], in_=wf2_t[:])
            wks = cpool.tile([128, NK * C * J], F16)
            nc.sync.dma_start(out=wks[:], in_=wks_t[:])

            rawh_v = rawh_t.ap().rearrange("c (j n) -> (c j) n", j=J)
            out0_v = out0_t.ap()

            # super s = 1024 cols = two 512-col PSUM-bank halves
            rhs_tiles = {}
            fps_tiles = {}
            g_tiles = {}
            feat_tiles = {}
            ops_tiles = {}
            NS = NJ // 1024  # 64

            def emit_dma_in(s):
                if s >= NS:
                    return
                n0 = s * 1024
                rhs2 = iopool.tile([96, CH], F16, tag="rhs")
                nc.sync.dma_start(out=rhs2[:48], in_=rawh_v[:, n0 : n0 + CH])
                nc.sync.dma_start(
                    out=rhs2[48:96], in_=rawh_v[:, n0 + CH : n0 + 2 * CH]
                )
                rhs_tiles[s] = rhs2

            def emit_proj(s):
                if s >= NS:
                    return
                rhs2 = rhs_tiles.pop(s)
                fps = pfpool.tile([128, 1024], F32, tag="fps")
                for h in range(2):
                    sl = slice(h * CH, (h + 1) * CH)
                    nc.tensor.matmul(
                        fps[0:64, sl], wf2[:, 128 * h : 128 * h + 64], rhs2[:],
                        start=True, stop=True, tile_position=(0, 0),
                    )
                    nc.tensor.matmul(
                        fps[64:128, sl], wf2[:, 128 * h + 64 : 128 * h + 128],
                        rhs2[:], start=True, stop=True, tile_position=(0, 64),
                    )
                fps_tiles[s] = fps

            def emit_g(s):
                if s >= NS:
                    return
                fps = fps_tiles.pop(s)
                g = gpool.tile([128, 1024], F16, tag="g")
                nc.scalar.activation(
                    g[:], fps[:], Ident, bias=pT[:, 1:2], scale=pT[:, 0:1]
                )
                g_tiles[s] = g

            def emit_dve_feats(s):
                if s >= NS:
                    return
                g = g_tiles[s]
                feats, korder = [g], [0]
                for j in DVE_KNOTS:
                    Rk = rpool.tile([128, 1024], F16, tag="R")
                    nc.vector.tensor_scalar(
                        out=Rk[:], in0=g[:], scalar1=float(j) - GSH,
                        scalar2=None, op0=mn,
                    )
                    feats.append(Rk)
                    korder.append(j)
                feat_tiles[s] = (feats, korder)

            def emit_act_feats(s):
                if s >= NS:
                    return
                g = g_tiles.pop(s)
                feats, korder = feat_tiles[s]
                for i, j in enumerate(ACT_KNOTS):
                    Rk = rpool.tile([128, 1024], F16, tag="R")
                    nc.scalar.activation(
                        Rk[:], g[:], Relu, bias=pT[:, 2 + i : 3 + i], scale=-1.0
                    )
                    feats.append(Rk)
                    korder.append(j)

            def emit_knot_mms(s):
                if s >= NS:
                    return
                feats, korder = feat_tiles.pop(s)
                ops = popool.tile([128, 1024], F32, tag="ops")
                # knot-major: both 512-col halves of a knot run back-to-back
                # on the same tile (one weight load per knot); per-super
                # parity flip pairs the odd 17th knot across supers
                for ki in range(NK):
                    k = korder[ki]
                    wk = wks[:, k * C * J : (k + 1) * C * J]
                    base = 64 * ((ki + s) % 2)
                    for h in range(2):
                        sl = slice(h * CH, (h + 1) * CH)
                        nc.tensor.matmul(
                            ops[base : base + C * J, sl], wk, feats[ki][:, sl],
                            start=(ki < 2), stop=(ki >= NK - 2),
                            tile_position=(0, base),
                        )
                ops_tiles[s] = ops

            def emit_drain(s):
                if s < 0 or s >= NS:
                    return
                ops = ops_tiles.pop(s)
                ob = obpool.tile([OPART, 1024], F16, tag="ob")
                nc.scalar.copy(ob[:], ops[0:OPART])
                nc.sync.dma_start(
                    out=out0_v[:, s * 1024 : (s + 1) * 1024], in_=ob[:]
                )

            emit_dma_in(0)
            emit_dma_in(1)
            emit_proj(0)
            emit_g(0)
            for s in range(NS):
                emit_dma_in(s + 2)
                emit_proj(s + 1)
                emit_g(s + 1)
                emit_dve_feats(s)
                emit_act_feats(s)
                emit_knot_mms(s)
                emit_drain(s - 1)
            emit_drain(NS - 1)
    nc.compile()
    return nc


def _host_params(raw, ys, A):
    in_maps = []
    for b in range(B):
        Ab = A[b].astype(np.float32)
        mins = np.minimum(Ab, 0).sum(axis=0)
        maxs = np.maximum(Ab, 0).sum(axis=0)
        pinv = np.linalg.pinv(Ab).astype(np.float32)
        dxf = ((maxs + np.float32(EPS) - mins) / np.float32(NK)).astype(np.float32)
        inv_dx = (np.float32(1.0) / dxf).astype(np.float32)
        off = (-mins * inv_dx).astype(np.float32)
        Y = np.concatenate(
            [mins[:, None], ys[b].astype(np.float32), maxs[:, None]], axis=1
        )
        s = np.diff(Y, axis=1).astype(np.float32)
        coef = np.empty((NA, NK), np.float32)
        coef[:, 0] = s[:, NK - 1]
        coef[:, 1:] = s[:, :-1] - s[:, 1:]

        par = np.zeros((128, 2 + len(ACT_KNOTS)), np.float32)
        par[:, 0] = np.repeat(inv_dx, J)
        par[:, 1] = np.repeat(off, J) - np.float32(GSH)
        for i, j in enumerate(ACT_KNOTS):
            par[:, 2 + i] = np.float32(j) - np.float32(GSH)

        wf = np.zeros((C, J, NA, J), np.float32)
        for j in range(J):
            wf[:, j, :, j] = Ab
        wf = wf.reshape(C * J, NA * J)
        wf2 = np.zeros((96, 2, 128), np.float32)
        wf2[0:48, 0] = wf
        wf2[48:96, 1] = wf
        wf2 = wf2.reshape(96, 2 * 128).astype(np.float16)

        csgn = coef.copy()
        for j in ACT_KNOTS:
            csgn[:, j] = -coef[:, j]
        wks = np.zeros((NA, J, NK, C, J), np.float32)
        for j in range(J):
            wks[:, j, :, :, j] = csgn[:, :, None] * pinv[:, None, :]
        wks = wks.reshape(NA * J, NK * C * J).astype(np.float16)

        b0 = (pinv * Y[:, 0:1]).sum(axis=0).astype(np.float32)
        b0 = b0 + np.float32(GSH) * (pinv * coef[:, 0:1]).sum(axis=0)
        for j in range(1, NK):
            sh = np.float32(j) if j in ACT_KNOTS else np.float32(GSH)
            b0 = b0 + sh * (pinv * coef[:, j : j + 1]).sum(axis=0)

        rh = np.ascontiguousarray(raw[b].reshape(C, HW)).astype(np.float16)
        in_maps.append({"rawh": rh, "par": par, "wf2": wf2, "wks": wks, "_b0": b0})
    return in_maps


def kernel(raw, ys, A):
    raw = np.asarray(raw, np.float32)
    ys = np.asarray(ys, np.float32)
    A = np.asarray(A, np.float32)
    if "nc" not in _NC_CACHE:
        _NC_CACHE["nc"] = _build_nc()
    nc = _NC_CACHE["nc"]
    in_maps = _host_params(raw, ys, A)
    dev_maps = [{k: v for k, v in m.items() if not k.startswith("_")} for m in in_maps]
    res = run_bass_kernel_spmd(nc, dev_maps, core_ids=list(range(B)))
    outs = []
    for b in range(B):
        slab = res.results[b]["out0"].astype(np.float32)
        if COL_TILE:
            o = slab[0 : C * J] + slab[64 : 64 + C * J]
        else:
            o = slab[0 : C * J]
        b0 = in_maps[b]["_b0"]
        o = o.reshape(C, J, NJ) + b0[:, None, None]
        outs.append(o.reshape(C, H, W))
    return np.stack(outs).astype(np.float32)
